# revision 8
# baseline (speedup 1.0000x reference)
"""Trainium2 Bass kernel for CPMAttention (GQA attention block).

Sharding: 8 cores = DP2 (batch) x TP4 (heads). Each core handles one batch
element and 8 q heads / 2 kv heads. w_qkv column-sharded, w_out
column-sharded; 4-rank AllGather of transposed attention outputs between
attention and out-projection.
"""
import sys
import numpy as np

for _p in ("/opt/trn_rl_repo", "/root/.axon_site/_ro/trn_rl_repo"):
    if _p not in sys.path:
        sys.path.append(_p)

import concourse.bass as bass
import concourse.mybir as mybir
import concourse.tile as tile
from concourse.bass_utils import run_bass_kernel_spmd
from concourse.masks import make_identity

F32 = mybir.dt.float32
F32R = mybir.dt.float32r

B, S, H = 2, 1024, 4096
NH, NKV, HD = 32, 8, 128
TP = 4
NH_L, NKV_L = NH // TP, NKV // TP            # 8 q heads, 2 kv heads per core
QL, KL, VL = NH_L * HD, NKV_L * HD, NKV_L * HD   # 1024, 256, 256
FL = QL + KL + VL                            # 1536 local qkv columns
OL = H // TP                                 # 1024 output columns per core
EPS = 1e-6
THETA = 10000.0
SCALE = HD ** -0.5

_CACHED = None


def _build():
    nc = bass.Bass(num_devices=8, name="cpm_attention")

    hidden = nc.dram_tensor("hidden", [S, H], F32, kind="ExternalInput")
    w_qkv = nc.dram_tensor("w_qkv", [H, FL], F32R, kind="ExternalInput")
    w_out = nc.dram_tensor("w_out", [H, OL], F32R, kind="ExternalInput")
    ropet = nc.dram_tensor("ropet", [4, 128, S], F32, kind="ExternalInput")
    out = nc.dram_tensor("out", [S, OL], F32, kind="ExternalOutput")

    # bounce buffers for the per-s-half AllGather (concat on dim0, rank-major)
    attn_loc = [nc.dram_tensor(f"attn_loc{i}", [QL, 512], F32R, kind="Internal")
                for i in range(2)]
    attn_gat = [nc.dram_tensor(f"attn_gat{i}", [TP * QL, 512], F32R, kind="Internal")
                for i in range(2)]
    groups = [[0, 1, 2, 3], [4, 5, 6, 7]]

    with tile.TileContext(nc) as tc:
        import contextlib
        with contextlib.ExitStack() as ctx:
            consts = ctx.enter_context(tc.tile_pool(name="consts", bufs=1))
            probs_p = ctx.enter_context(tc.tile_pool(name="probs", bufs=3))
            acc_p = ctx.enter_context(tc.tile_pool(name="acc", bufs=2))
            ao_p = ctx.enter_context(tc.tile_pool(name="ao", bufs=2))

            ident = consts.tile([128, 128], F32)
            make_identity(nc, ident)
            ones_f = consts.tile([128, 128], F32)
            nc.vector.memset(ones_f, 1.0)
            ones_r = consts.tile([128, 128], F32R)
            nc.vector.tensor_copy(ones_r, ones_f)
            # causal wedge mask: 0 where i<=j else -1e30
            eps_t = consts.tile([128, 1], F32)
            nc.vector.memset(eps_t, EPS)
            maskt = consts.tile([128, 128], F32)
            nc.gpsimd.memset(maskt, 0.0)
            nc.gpsimd.affine_select(
                out=maskt, in_=maskt,
                compare_op=mybir.AluOpType.is_ge,
                fill=-1e30, base=0,
                pattern=[[1, 128]], channel_multiplier=-1,
            )
            def rms_rope_transpose(ps_src, col, gst, dst, dslot, tq):
                """ps_src[:, col:col+128] (tokens x head_dim) -> normalized,
                transposed into dst[:, dslot, gst*128:(gst+1)*128]."""
                scratch = norm_p.tile([128, 128], F32, tag="scratch")
                ssum = rstd_p.tile([128, 1], F32, tag="ssum")
                nc.scalar.activation(scratch, ps_src[:, col:col + 128],
                                     mybir.ActivationFunctionType.Square,
                                     accum_out=ssum)
                rstd = rstd_p.tile([128, 1], F32, tag="rstd")
                nc.scalar.activation(rstd, ssum,
                                     mybir.ActivationFunctionType.Sqrt,
                                     scale=1.0 / HD, bias=eps_t)
                nc.vector.reciprocal(rstd, rstd)
                qn = norm_p.tile([128, 128], F32, tag="qn")
                nc.scalar.activation(qn, ps_src[:, col:col + 128],
                                     mybir.ActivationFunctionType.Copy,
                                     scale=rstd)
                tps = aux_ps.tile([128, 128], F32, tag="aux")
                nc.tensor.transpose(tps, qn, ident)
                nc.any.tensor_copy(dst[:, dslot, gst * 128:(gst + 1) * 128], tps)

            def rope_apply(dst_slice, base, s0):
                # dst_slice: [128, 512] f32r (d on partitions); base 0 for q, 2 for k
                # table slots: [base]: rows 0:64 = cos*w1 (T0), 64:128 = sin*w2 (T1)
                #              [base+1]: rows 0:64 = sin*w1 (T3), 64:128 = cos*w2 (T2)
                x1 = dst_slice[0:64, :]
                x2 = dst_slice[64:128, :]
                t0 = ropes[0:64, base, s0:s0 + 512]
                t1 = ropes[64:128, base, s0:s0 + 512]
                t3 = ropes[0:64, base + 1, s0:s0 + 512]
                t2 = ropes[64:128, base + 1, s0:s0 + 512]
                a = ropetmp_p.tile([64, 512], F32, tag="a")
                b = ropetmp_p.tile([64, 512], F32, tag="b")
                c = ropetmp_p.tile([64, 512], F32, tag="c")
                d = ropetmp_p.tile([64, 512], F32, tag="d")
                nc.vector.tensor_mul(a, x1, t0)
                nc.vector.tensor_mul(b, x2, t1)
                nc.vector.tensor_mul(c, x2, t2)
                nc.vector.tensor_mul(d, x1, t3)
                nc.vector.tensor_tensor(x1, a, b, mybir.AluOpType.subtract)
                nc.vector.tensor_tensor(x2, c, d, mybir.AluOpType.add)

            with tc.tile_pool(name="ropes", bufs=1) as ropes_p, \
                 tc.tile_pool(name="qt", bufs=1) as qt_p, \
                 tc.tile_pool(name="kt", bufs=1) as kt_p, \
                 tc.tile_pool(name="v", bufs=1) as v_p, \
                 tc.tile_pool(name="norm", bufs=3) as norm_p, \
                 tc.tile_pool(name="rstd", bufs=4) as rstd_p, \
                 tc.tile_pool(name="ropetmp", bufs=4) as ropetmp_p, \
                 tc.tile_pool(name="stage", bufs=2) as stage_p, \
                 tc.tile_pool(name="ht", bufs=1) as ht_p, \
                 tc.tile_pool(name="wstream", bufs=3) as w_p, \
                 tc.tile_pool(name="aux_ps", bufs=2, space="PSUM") as aux_ps, \
                 tc.tile_pool(name="big_ps", bufs=2, space="PSUM") as big_ps, \
                 tc.tile_pool(name="sc_ps", bufs=2, space="PSUM") as sc_ps:
                # rope tables: [128, 4, S]; rows 0:64 q tables, 64:128 k tables
                ropes = ropes_p.tile([128, 4, S], F32)
                for t in range(4):
                    nc.sync.dma_start(ropes[:, t, :], ropet[t])
                qT = qt_p.tile([128, NH_L, S], F32R)
                kT = kt_p.tile([128, NKV_L, S], F32R)
                v_sb = v_p.tile([128, 8, VL], F32R)   # [sk-part, s-tile, 2*HD]
                for sb in range(2):
                    s0 = sb * 512
                    hT = ht_p.tile([128, 32, 512], F32R, tag="ht")
                    # A. transpose hidden[s0:s0+512, :] into hT
                    for st in range(4):
                        for q4 in range(4):
                            hs = stage_p.tile([128, 1024], F32, tag="hs")
                            nc.sync.dma_start(
                                hs, hidden[s0 + st * 128:s0 + (st + 1) * 128,
                                           q4 * 1024:(q4 + 1) * 1024])
                            for hb in range(8):
                                ht_idx = q4 * 8 + hb
                                tps = aux_ps.tile([128, 128], F32, tag="aux")
                                nc.tensor.transpose(
                                    tps, hs[:, hb * 128:(hb + 1) * 128], ident)
                                nc.any.tensor_copy(
                                    hT[:, ht_idx, st * 128:(st + 1) * 128], tps)
                    # B/C. QKV matmuls + per-head postprocess
                    for fb in range(3):
                        pss2 = [big_ps.tile([128, 2, 512], F32, tag="big", name=f"qkv_ps{i}") for i in range(2)]
                        pss = [pss2[i // 2][:, i % 2, :] for i in range(4)]
                        for h in range(32):
                            wt = w_p.tile([128, 512], F32R, tag="w")
                            nc.sync.dma_start(
                                wt, w_qkv[h * 128:(h + 1) * 128,
                                          fb * 512:(fb + 1) * 512])
                            for st in range(4):
                                nc.tensor.matmul(
                                    pss[st], hT[:, h, st * 128:(st + 1) * 128], wt,
                                    start=(h == 0), stop=(h == 31))
                        for st in range(4):
                            gst = sb * 4 + st
                            if fb < 2:
                                for hq in range(4):
                                    head = fb * 4 + hq
                                    rms_rope_transpose(pss[st], hq * 128, gst,
                                                       qT, head, True)
                            else:
                                for kh in range(NKV_L):
                                    rms_rope_transpose(pss[st], kh * 128, gst,
                                                       kT, kh, False)
                                nc.any.tensor_copy(v_sb[:, gst, :],
                                                   pss[st][:, KL:KL + VL])
                    # D. rope on this s-half
                    for head in range(NH_L):
                        rope_apply(qT[:, head, s0:s0 + 512], 0, s0)
                    for kh in range(NKV_L):
                        rope_apply(kT[:, kh, s0:s0 + 512], 2, s0)
                    # E. attention for sq block sb, all heads
                    n_t = 4 * (sb + 1)
                    for head in range(NH_L):
                        kvh = head // 4
                        pv_t = big_ps.tile([128, 2, 512], F32, tag="big", name="pv")
                        pv = pv_t[:, 0, :]
                        acc = acc_p.tile([128, 512], F32R, tag="acc")
                        for t in range(n_t):
                            k0 = t * 128
                            w0 = max(0, k0 - s0)
                            sc = sc_ps.tile([128, 512], F32, tag="sc")
                            nc.tensor.matmul(
                                sc[:, w0:512],
                                kT[:, kvh, k0:k0 + 128],
                                qT[:, head, s0 + w0:s0 + 512],
                                start=True, stop=True)
                            if k0 >= s0:
                                nc.vector.tensor_add(
                                    sc[:, w0:w0 + 128], sc[:, w0:w0 + 128], maskt)
                            pr = probs_p.tile([128, 512], F32R, tag="pr")
                            nc.scalar.activation(
                                pr[:, w0:512], sc[:, w0:512],
                                mybir.ActivationFunctionType.Exp, scale=SCALE)
                            if t == 0:
                                nc.vector.tensor_copy(acc, pr)
                            else:
                                nc.vector.tensor_add(acc[:, w0:512],
                                                     acc[:, w0:512], pr[:, w0:512])
                            nc.tensor.matmul(
                                pv[:, w0:512],
                                v_sb[:, t, kvh * 128:(kvh + 1) * 128],
                                pr[:, w0:512],
                                start=(t == 0), stop=(t == n_t - 1))
                        lps = sc_ps.tile([128, 512], F32, tag="sc", name="lps")
                        nc.tensor.matmul(lps, ones_r, acc, start=True, stop=True)
                        rl = ao_p.tile([128, 512], F32, tag="rl")
                        nc.vector.reciprocal(rl, lps)
                        ao = ao_p.tile([128, 512], F32R, tag="ao")
                        nc.vector.tensor_mul(ao, pv, rl)
                        nc.gpsimd.dma_start(
                            attn_loc[sb][head * 128:(head + 1) * 128, :], ao)
                    # F. AllGather for this s-half
                    nc.gpsimd.collective_compute(
                        "AllGather", mybir.AluOpType.bypass,
                        ins=[attn_loc[sb][:, :]], outs=[attn_gat[sb][:, :]],
                        replica_groups=groups)

            # G. out-projection: out[s, o] = sum_f attn_T[f, s] * w_out[f, o]
            with tc.tile_pool(name="wout", bufs=1) as wout_p, \
                 tc.tile_pool(name="ag", bufs=3) as ag_p, \
                 tc.tile_pool(name="osb", bufs=2) as osb_p, \
                 tc.tile_pool(name="op_ps", bufs=2, space="PSUM") as op_ps:
                wout_sb = wout_p.tile([128, 32, OL], F32R)
                for ft in range(32):
                    nc.sync.dma_start(wout_sb[:, ft, :],
                                      w_out[ft * 128:(ft + 1) * 128, :])
                for gst in range(8):
                    chunk, stc = gst // 4, gst % 4
                    pso = [op_ps.tile([128, 512], F32, tag="op", name=f"op_ps{i}") for i in range(2)]
                    for ft in range(32):
                        agt = ag_p.tile([128, 128], F32R, tag="agt")
                        nc.sync.dma_start(
                            agt, attn_gat[chunk][ft * 128:(ft + 1) * 128,
                                                 stc * 128:(stc + 1) * 128])
                        for ob in range(2):
                            nc.tensor.matmul(
                                pso[ob], agt, wout_sb[:, ft, ob * 512:(ob + 1) * 512],
                                start=(ft == 0), stop=(ft == 31))
                    for ob in range(2):
                        osb = osb_p.tile([128, 512], F32, tag="osb")
                        nc.any.tensor_copy(osb, pso[ob])
                        nc.gpsimd.dma_start(
                            out[gst * 128:(gst + 1) * 128, ob * 512:(ob + 1) * 512],
                            osb)

    _split_excess_waits(nc)
    return nc


def _split_excess_waits(nc):
    # this walrus build allows only one semaphore wait per instruction;
    # hoist extras onto same-engine NOPs placed just before the instruction
    f = nc.m.functions[0]
    for bb in f.blocks:
        new_insts = []
        for inst in bb.instructions:
            si = inst.sync_info
            if si and si.on_wait and len(si.on_wait) > 1:
                extra = si.on_wait[:-1]
                si.on_wait[:] = si.on_wait[-1:]
                for w in extra:
                    nop = mybir.InstNoOp(
                        name=nc.get_next_instruction_name(), ins=[], outs=[],
                        engine=inst.engine,
                        sync_info=mybir.SyncInfo(on_wait=[w], on_update=[]))
                    nc.register_instruction(nop)
                    new_insts.append(nop)
            new_insts.append(inst)
        bb.instructions[:] = new_insts


def _rope_tables(positions_b, norm_w):
    # two slots [2, 128, S]: slot0 = [cos*w1 ; sin*w2], slot1 = [sin*w1 ; cos*w2]
    inv_freq = (1.0 / (THETA ** (np.arange(0, HD, 2, dtype=np.float32) / HD))
                ).astype(np.float32)
    ang = positions_b.astype(np.float32)[:, None] * inv_freq[None, :]
    cos = np.cos(ang).astype(np.float32)      # [S, 64]
    sin = np.sin(ang).astype(np.float32)
    w1 = norm_w[:64].astype(np.float32)
    w2 = norm_w[64:].astype(np.float32)
    slot0 = np.concatenate([(cos * w1[None, :]).T, (sin * w2[None, :]).T], axis=0)
    slot1 = np.concatenate([(sin * w1[None, :]).T, (cos * w2[None, :]).T], axis=0)
    return np.stack([slot0, slot1], axis=0).astype(np.float32)  # [2, 128, S]


def kernel(hidden_states, positions, w_qkv, w_out, q_norm_w, k_norm_w):
    global _CACHED
    if _CACHED is None:
        _CACHED = _build()
    nc = _CACHED

    in_maps = []
    for core in range(8):
        b, g = core // 4, core % 4
        w_local = np.ascontiguousarray(np.concatenate([
            w_qkv[:, g * QL:(g + 1) * QL],
            w_qkv[:, NH * HD + g * KL:NH * HD + (g + 1) * KL],
            w_qkv[:, NH * HD + NKV * HD + g * VL:NH * HD + NKV * HD + (g + 1) * VL],
        ], axis=1)).astype(np.float32)
        wout_local = np.ascontiguousarray(
            w_out[:, g * OL:(g + 1) * OL]).astype(np.float32)
        tq = _rope_tables(np.asarray(positions[b]), np.asarray(q_norm_w))
        tk = _rope_tables(np.asarray(positions[b]), np.asarray(k_norm_w))
        ropet = np.concatenate([tq, tk], axis=0)  # [4, 128, S]
        in_maps.append({
            "hidden": np.ascontiguousarray(hidden_states[b]).astype(np.float32),
            "w_qkv": w_local,
            "w_out": wout_local,
            "ropet": np.ascontiguousarray(ropet),
        })

    res = run_bass_kernel_spmd(nc, in_maps, core_ids=list(range(8)))

    outp = np.empty((B, S, H), dtype=np.float32)
    for core in range(8):
        b, g = core // 4, core % 4
        outp[b, :, g * OL:(g + 1) * OL] = res.results[core]["out"]
    return outp


# revision 9
# speedup vs baseline: 1.0388x; 1.0388x over previous
"""Trainium2 Bass kernel for CPMAttention (GQA attention block).

Sharding: 8 cores = DP2 (batch) x TP4 (heads). Each core handles one batch
element and 8 q heads / 2 kv heads. w_qkv column-sharded, w_out
column-sharded; per-s-half 4-rank AllGather of transposed attention outputs
between attention and out-projection.

Compute: bf16 matmul operands with fp32 PSUM accumulation; softmax and
norms in fp32 (no row-max subtraction: RMS-normed q,k bound |logits| <= 11.4).
"""
import sys
import numpy as np
import ml_dtypes

for _p in ("/opt/trn_rl_repo", "/root/.axon_site/_ro/trn_rl_repo"):
    if _p not in sys.path:
        sys.path.append(_p)

import concourse.bass as bass
import concourse.mybir as mybir
import concourse.tile as tile
from concourse.bass_utils import run_bass_kernel_spmd
from concourse.masks import make_identity

F32 = mybir.dt.float32
BF16 = mybir.dt.bfloat16

B, S, H = 2, 1024, 4096
NH, NKV, HD = 32, 8, 128
TP = 4
NH_L, NKV_L = NH // TP, NKV // TP            # 8 q heads, 2 kv heads per core
QL, KL, VL = NH_L * HD, NKV_L * HD, NKV_L * HD   # 1024, 256, 256
FL = QL + KL + VL                            # 1536 local qkv columns
OL = H // TP                                 # 1024 output columns per core
EPS = 1e-6
THETA = 10000.0
SCALE = HD ** -0.5

_CACHED = None


def _build():
    nc = bass.Bass(num_devices=8, name="cpm_attention")

    hidden = nc.dram_tensor("hidden", [S, H], BF16, kind="ExternalInput")
    w_qkv = nc.dram_tensor("w_qkv", [H, FL], BF16, kind="ExternalInput")
    w_out = nc.dram_tensor("w_out", [H, OL], BF16, kind="ExternalInput")
    ropet = nc.dram_tensor("ropet", [4, 128, S], F32, kind="ExternalInput")
    out = nc.dram_tensor("out", [S, OL], F32, kind="ExternalOutput")

    # bounce buffers for the per-s-half AllGather (concat on dim0, rank-major)
    attn_loc = [nc.dram_tensor(f"attn_loc{i}", [QL, 512], BF16, kind="Internal")
                for i in range(2)]
    attn_gat = [nc.dram_tensor(f"attn_gat{i}", [TP * QL, 512], BF16, kind="Internal")
                for i in range(2)]
    groups = [[0, 1, 2, 3], [4, 5, 6, 7]]

    with tile.TileContext(nc) as tc:
        import contextlib
        with contextlib.ExitStack() as ctx:
            consts = ctx.enter_context(tc.tile_pool(name="consts", bufs=1))
            wout_p = ctx.enter_context(tc.tile_pool(name="wout", bufs=1))
            probs_p = ctx.enter_context(tc.tile_pool(name="probs", bufs=4))
            acc_p = ctx.enter_context(tc.tile_pool(name="acc", bufs=2))
            ao_p = ctx.enter_context(tc.tile_pool(name="ao", bufs=2))

            ident = consts.tile([128, 128], BF16)
            make_identity(nc, ident)
            ones_b = consts.tile([128, 128], BF16)
            nc.vector.memset(ones_b, 1.0)
            eps_t = consts.tile([128, 1], F32)
            nc.vector.memset(eps_t, EPS)
            # causal wedge mask: 0 where i<=j else -1e30
            maskt = consts.tile([128, 128], F32)
            nc.gpsimd.memset(maskt, 0.0)
            nc.gpsimd.affine_select(
                out=maskt, in_=maskt,
                compare_op=mybir.AluOpType.is_ge,
                fill=-1e30, base=0,
                pattern=[[1, 128]], channel_multiplier=-1,
            )

            # w_out resident, loaded up-front (bf16, 64KB/partition)
            wout_sb = wout_p.tile([128, 32, OL], BF16)
            for ft in range(32):
                nc.sync.dma_start(wout_sb[:, ft, :],
                                  w_out[ft * 128:(ft + 1) * 128, :])

            with tc.tile_pool(name="ropes", bufs=1) as ropes_p, \
                 tc.tile_pool(name="qt", bufs=1) as qt_p, \
                 tc.tile_pool(name="kt", bufs=1) as kt_p, \
                 tc.tile_pool(name="v", bufs=1) as v_p, \
                 tc.tile_pool(name="norm", bufs=3) as norm_p, \
                 tc.tile_pool(name="rstd", bufs=4) as rstd_p, \
                 tc.tile_pool(name="ropetmp", bufs=4) as ropetmp_p, \
                 tc.tile_pool(name="ht", bufs=1) as ht_p, \
                 tc.tile_pool(name="wstream", bufs=3) as w_p, \
                 tc.tile_pool(name="aux_ps", bufs=1, space="PSUM") as aux_ps, \
                 tc.tile_pool(name="big_ps", bufs=2, space="PSUM") as big_ps, \
                 tc.tile_pool(name="sc_ps", bufs=2, space="PSUM") as sc_ps, \
                 tc.tile_pool(name="l_ps", bufs=1, space="PSUM") as l_ps:
                # rope tables: [128, 4, S]; rows 0:64 x1-tables, 64:128 x2-tables
                ropes = ropes_p.tile([128, 4, S], F32)
                for t in range(4):
                    nc.sync.dma_start(ropes[:, t, :], ropet[t])
                qT = qt_p.tile([128, NH_L, S], BF16)
                kT = kt_p.tile([128, NKV_L, S], BF16)
                v_sb = v_p.tile([128, 8, VL], BF16)   # [sk-part, s-tile, 2*HD]

                def rms_rope_transpose(ps_src, col, gst, dst, dslot):
                    """ps_src[:, col:col+128] (tokens x head_dim, fp32 psum) ->
                    RMS-normalized, transposed into dst[:, dslot, gst-slice]."""
                    scratch = norm_p.tile([128, 128], F32, tag="scratch")
                    ssum = rstd_p.tile([128, 1], F32, tag="ssum")
                    nc.scalar.activation(scratch, ps_src[:, col:col + 128],
                                         mybir.ActivationFunctionType.Square,
                                         accum_out=ssum)
                    rstd = rstd_p.tile([128, 1], F32, tag="rstd")
                    nc.scalar.activation(rstd, ssum,
                                         mybir.ActivationFunctionType.Sqrt,
                                         scale=1.0 / HD, bias=eps_t)
                    nc.vector.reciprocal(rstd, rstd)
                    qn = norm_p.tile([128, 128], BF16, tag="qn")
                    nc.scalar.activation(qn, ps_src[:, col:col + 128],
                                         mybir.ActivationFunctionType.Copy,
                                         scale=rstd)
                    tps = aux_ps.tile([128, 128], BF16, tag="aux")
                    nc.tensor.transpose(tps, qn, ident)
                    nc.any.tensor_copy(dst[:, dslot, gst * 128:(gst + 1) * 128], tps)

                def rope_apply(dst_slice, base, s0):
                    # dst_slice: [128, 512] bf16 (d on partitions); base 0=q, 2=k
                    # slots: [base]: rows 0:64 = cos*w1, 64:128 = sin*w2
                    #        [base+1]: rows 0:64 = sin*w1, 64:128 = cos*w2
                    x1 = dst_slice[0:64, :]
                    x2 = dst_slice[64:128, :]
                    t0 = ropes[0:64, base, s0:s0 + 512]
                    t1 = ropes[64:128, base, s0:s0 + 512]
                    t3 = ropes[0:64, base + 1, s0:s0 + 512]
                    t2 = ropes[64:128, base + 1, s0:s0 + 512]
                    a = ropetmp_p.tile([64, 512], F32, tag="a")
                    b = ropetmp_p.tile([64, 512], F32, tag="b")
                    c = ropetmp_p.tile([64, 512], F32, tag="c")
                    d = ropetmp_p.tile([64, 512], F32, tag="d")
                    nc.vector.tensor_mul(a, x1, t0)
                    nc.vector.tensor_mul(b, x2, t1)
                    nc.vector.tensor_mul(c, x2, t2)
                    nc.vector.tensor_mul(d, x1, t3)
                    nc.vector.tensor_tensor(x1, a, b, mybir.AluOpType.subtract)
                    nc.vector.tensor_tensor(x2, c, d, mybir.AluOpType.add)

                for sb in range(2):
                    s0 = sb * 512
                    hT = ht_p.tile([128, 32, 512], BF16, tag="ht")
                    # A. DMA-transpose hidden[s0:s0+512, :] into hT (bf16 XBAR)
                    for st in range(4):
                        for ht_idx in range(32):
                            nc.sync.dma_start(
                                hT[:, ht_idx, st * 128:(st + 1) * 128],
                                hidden[s0 + st * 128:s0 + (st + 1) * 128,
                                       ht_idx * 128:(ht_idx + 1) * 128],
                                transpose=True)
                    # B/C. QKV matmuls + per-head postprocess
                    for fb in range(3):
                        pss2 = [big_ps.tile([128, 2, 512], F32, tag="big",
                                            name=f"qkv_ps{i}") for i in range(2)]
                        pss = [pss2[i // 2][:, i % 2, :] for i in range(4)]
                        for h in range(32):
                            wt = w_p.tile([128, 512], BF16, tag="w")
                            nc.sync.dma_start(
                                wt, w_qkv[h * 128:(h + 1) * 128,
                                          fb * 512:(fb + 1) * 512])
                            for st in range(4):
                                nc.tensor.matmul(
                                    pss[st], hT[:, h, st * 128:(st + 1) * 128], wt,
                                    start=(h == 0), stop=(h == 31))
                        for st in range(4):
                            gst = sb * 4 + st
                            if fb < 2:
                                for hq in range(4):
                                    rms_rope_transpose(pss[st], hq * 128, gst,
                                                       qT, fb * 4 + hq)
                            else:
                                for kh in range(NKV_L):
                                    rms_rope_transpose(pss[st], kh * 128, gst,
                                                       kT, kh)
                                nc.any.tensor_copy(v_sb[:, gst, :],
                                                   pss[st][:, KL:KL + VL])
                    # D. rope on this s-half
                    for head in range(NH_L):
                        rope_apply(qT[:, head, s0:s0 + 512], 0, s0)
                    for kh in range(NKV_L):
                        rope_apply(kT[:, kh, s0:s0 + 512], 2, s0)
                    # E. attention for sq block sb, all heads
                    n_t = 4 * (sb + 1)
                    for head in range(NH_L):
                        kvh = head // 4
                        pv_t = big_ps.tile([128, 2, 512], F32, tag="big", name="pv")
                        pv = pv_t[:, 0, :]
                        acc = acc_p.tile([128, 512], BF16, tag="acc")
                        for t in range(n_t):
                            k0 = t * 128
                            w0 = max(0, k0 - s0)
                            sc = sc_ps.tile([128, 512], F32, tag="sc")
                            nc.tensor.matmul(
                                sc[:, w0:512],
                                kT[:, kvh, k0:k0 + 128],
                                qT[:, head, s0 + w0:s0 + 512],
                                start=True, stop=True)
                            if k0 >= s0:
                                nc.vector.tensor_add(
                                    sc[:, w0:w0 + 128], sc[:, w0:w0 + 128], maskt)
                            pr = probs_p.tile([128, 512], BF16, tag="pr")
                            nc.scalar.activation(
                                pr[:, w0:512], sc[:, w0:512],
                                mybir.ActivationFunctionType.Exp, scale=SCALE)
                            if t == 0:
                                nc.vector.tensor_copy(acc, pr)
                            else:
                                nc.vector.tensor_add(acc[:, w0:512],
                                                     acc[:, w0:512], pr[:, w0:512])
                            nc.tensor.matmul(
                                pv[:, w0:512],
                                v_sb[:, t, kvh * 128:(kvh + 1) * 128],
                                pr[:, w0:512],
                                start=(t == 0), stop=(t == n_t - 1))
                        lps = l_ps.tile([128, 512], F32, tag="l")
                        nc.tensor.matmul(lps, ones_b, acc, start=True, stop=True)
                        rl = ao_p.tile([128, 512], F32, tag="rl")
                        nc.vector.reciprocal(rl, lps)
                        ao = ao_p.tile([128, 512], BF16, tag="ao")
                        nc.vector.tensor_mul(ao, pv, rl)
                        nc.gpsimd.dma_start(
                            attn_loc[sb][head * 128:(head + 1) * 128, :], ao)
                    # F. AllGather for this s-half
                    nc.gpsimd.collective_compute(
                        "AllGather", mybir.AluOpType.bypass,
                        ins=[attn_loc[sb][:, :]], outs=[attn_gat[sb][:, :]],
                        replica_groups=groups)

            # G. out-projection: out[s, o] = sum_f attn_T[f, s] * w_out[f, o]
            with tc.tile_pool(name="ag", bufs=3) as ag_p, \
                 tc.tile_pool(name="osb", bufs=2) as osb_p, \
                 tc.tile_pool(name="op_ps", bufs=2, space="PSUM") as op_ps:
                for gst in range(8):
                    chunk, stc = gst // 4, gst % 4
                    pso = [op_ps.tile([128, 512], F32, tag="op", name=f"op_ps{i}")
                           for i in range(2)]
                    for ft in range(32):
                        agt = ag_p.tile([128, 128], BF16, tag="agt")
                        nc.sync.dma_start(
                            agt, attn_gat[chunk][ft * 128:(ft + 1) * 128,
                                                 stc * 128:(stc + 1) * 128])
                        for ob in range(2):
                            nc.tensor.matmul(
                                pso[ob], agt, wout_sb[:, ft, ob * 512:(ob + 1) * 512],
                                start=(ft == 0), stop=(ft == 31))
                    for ob in range(2):
                        osb = osb_p.tile([128, 512], F32, tag="osb")
                        nc.any.tensor_copy(osb, pso[ob])
                        nc.gpsimd.dma_start(
                            out[gst * 128:(gst + 1) * 128, ob * 512:(ob + 1) * 512],
                            osb)

    _split_excess_waits(nc)
    return nc


def _split_excess_waits(nc):
    # this walrus build allows only one semaphore wait per instruction;
    # hoist extras onto same-engine NOPs placed just before the instruction
    f = nc.m.functions[0]
    for bb in f.blocks:
        new_insts = []
        for inst in bb.instructions:
            si = inst.sync_info
            if si and si.on_wait and len(si.on_wait) > 1:
                extra = si.on_wait[:-1]
                si.on_wait[:] = si.on_wait[-1:]
                for w in extra:
                    nop = mybir.InstNoOp(
                        name=nc.get_next_instruction_name(), ins=[], outs=[],
                        engine=inst.engine,
                        sync_info=mybir.SyncInfo(on_wait=[w], on_update=[]))
                    nc.register_instruction(nop)
                    new_insts.append(nop)
            new_insts.append(inst)
        bb.instructions[:] = new_insts


def _rope_tables(positions_b, norm_w):
    # two slots [2, 128, S]: slot0 = [cos*w1 ; sin*w2], slot1 = [sin*w1 ; cos*w2]
    inv_freq = (1.0 / (THETA ** (np.arange(0, HD, 2, dtype=np.float32) / HD))
                ).astype(np.float32)
    ang = positions_b.astype(np.float32)[:, None] * inv_freq[None, :]
    cos = np.cos(ang).astype(np.float32)      # [S, 64]
    sin = np.sin(ang).astype(np.float32)
    w1 = norm_w[:64].astype(np.float32)
    w2 = norm_w[64:].astype(np.float32)
    slot0 = np.concatenate([(cos * w1[None, :]).T, (sin * w2[None, :]).T], axis=0)
    slot1 = np.concatenate([(sin * w1[None, :]).T, (cos * w2[None, :]).T], axis=0)
    return np.stack([slot0, slot1], axis=0).astype(np.float32)  # [2, 128, S]


def kernel(hidden_states, positions, w_qkv, w_out, q_norm_w, k_norm_w):
    global _CACHED
    if _CACHED is None:
        _CACHED = _build()
    nc = _CACHED

    in_maps = []
    for core in range(8):
        b, g = core // 4, core % 4
        w_local = np.ascontiguousarray(np.concatenate([
            w_qkv[:, g * QL:(g + 1) * QL],
            w_qkv[:, NH * HD + g * KL:NH * HD + (g + 1) * KL],
            w_qkv[:, NH * HD + NKV * HD + g * VL:NH * HD + NKV * HD + (g + 1) * VL],
        ], axis=1)).astype(ml_dtypes.bfloat16)
        wout_local = np.ascontiguousarray(
            w_out[:, g * OL:(g + 1) * OL]).astype(ml_dtypes.bfloat16)
        tq = _rope_tables(np.asarray(positions[b]), np.asarray(q_norm_w))
        tk = _rope_tables(np.asarray(positions[b]), np.asarray(k_norm_w))
        ropet = np.concatenate([tq, tk], axis=0)  # [4, 128, S]
        in_maps.append({
            "hidden": np.ascontiguousarray(hidden_states[b]).astype(ml_dtypes.bfloat16),
            "w_qkv": w_local,
            "w_out": wout_local,
            "ropet": np.ascontiguousarray(ropet),
        })

    res = run_bass_kernel_spmd(nc, in_maps, core_ids=list(range(8)))

    outp = np.empty((B, S, H), dtype=np.float32)
    for core in range(8):
        b, g = core // 4, core % 4
        outp[b, :, g * OL:(g + 1) * OL] = res.results[core]["out"]
    return outp


# revision 11
# speedup vs baseline: 5.8316x; 5.6140x over previous
"""Trainium2 Bass kernel for CPMAttention (GQA attention block).

Sharding: 8 cores = DP2 (batch) x TP4 (heads). Each core handles one batch
element and 8 q heads / 2 kv heads. w_qkv column-sharded, w_out
column-sharded; per-s-half 4-rank AllGather of transposed attention outputs
between attention and out-projection.

Compute: bf16 matmul operands with fp32 PSUM accumulation; softmax and
norms in fp32 (no row-max subtraction: RMS-normed q,k bound |logits| <= 11.4).
"""
import sys
import numpy as np
import ml_dtypes

for _p in ("/opt/trn_rl_repo", "/root/.axon_site/_ro/trn_rl_repo"):
    if _p not in sys.path:
        sys.path.append(_p)

import concourse.bass as bass
import concourse.mybir as mybir
import concourse.tile as tile
from concourse.bass_utils import run_bass_kernel_spmd
from concourse.masks import make_identity

F32 = mybir.dt.float32
BF16 = mybir.dt.bfloat16

B, S, H = 2, 1024, 4096
NH, NKV, HD = 32, 8, 128
TP = 4
NH_L, NKV_L = NH // TP, NKV // TP            # 8 q heads, 2 kv heads per core
QL, KL, VL = NH_L * HD, NKV_L * HD, NKV_L * HD   # 1024, 256, 256
FL = QL + KL + VL                            # 1536 local qkv columns
OL = H // TP                                 # 1024 output columns per core
EPS = 1e-6
THETA = 10000.0
SCALE = HD ** -0.5

_CACHED = None


def _build():
    nc = bass.Bass(num_devices=8, name="cpm_attention")

    hidden = nc.dram_tensor("hidden", [S, H], BF16, kind="ExternalInput")
    w_qkv = nc.dram_tensor("w_qkv", [H, FL], BF16, kind="ExternalInput")
    w_out = nc.dram_tensor("w_out", [H, OL], BF16, kind="ExternalInput")
    ropet = nc.dram_tensor("ropet", [4, 128, S], F32, kind="ExternalInput")
    out = nc.dram_tensor("out", [S, OL], F32, kind="ExternalOutput")

    # bounce buffers for the per-s-half AllGather (concat on dim0, rank-major)
    attn_loc = [nc.dram_tensor(f"attn_loc{i}", [QL, 512], BF16, kind="Internal")
                for i in range(2)]
    attn_gat = [nc.dram_tensor(f"attn_gat{i}", [TP * QL, 512], BF16, kind="Internal")
                for i in range(2)]
    groups = [[0, 1, 2, 3], [4, 5, 6, 7]]

    with tile.TileContext(nc) as tc:
        import contextlib
        with contextlib.ExitStack() as ctx:
            consts = ctx.enter_context(tc.tile_pool(name="consts", bufs=1))
            wout_p = ctx.enter_context(tc.tile_pool(name="wout", bufs=1))
            probs_p = ctx.enter_context(tc.tile_pool(name="probs", bufs=4))
            acc_p = ctx.enter_context(tc.tile_pool(name="acc", bufs=2))
            ao_p = ctx.enter_context(tc.tile_pool(name="ao", bufs=2))

            ident = consts.tile([128, 128], BF16)
            make_identity(nc, ident)
            ones_b = consts.tile([128, 128], BF16)
            nc.vector.memset(ones_b, 1.0)
            eps_t = consts.tile([128, 1], F32)
            nc.vector.memset(eps_t, EPS)
            # causal wedge mask: 0 where i<=j else -1e30
            maskt = consts.tile([128, 128], F32)
            nc.gpsimd.memset(maskt, 0.0)
            nc.gpsimd.affine_select(
                out=maskt, in_=maskt,
                compare_op=mybir.AluOpType.is_ge,
                fill=-1e30, base=0,
                pattern=[[1, 128]], channel_multiplier=-1,
            )

            # w_out resident, loaded up-front (bf16, 64KB/partition)
            wout_sb = wout_p.tile([128, 32, OL], BF16)
            for ft in range(32):
                nc.scalar.dma_start(wout_sb[:, ft, :],
                                    w_out[ft * 128:(ft + 1) * 128, :])

            with tc.tile_pool(name="ropes", bufs=1) as ropes_p, \
                 tc.tile_pool(name="qt", bufs=1) as qt_p, \
                 tc.tile_pool(name="kt", bufs=1) as kt_p, \
                 tc.tile_pool(name="v", bufs=1) as v_p, \
                 tc.tile_pool(name="norm", bufs=3) as norm_p, \
                 tc.tile_pool(name="rstd", bufs=4) as rstd_p, \
                 tc.tile_pool(name="ropetmp", bufs=4) as ropetmp_p, \
                 tc.tile_pool(name="ht", bufs=1) as ht_p, \
                 tc.tile_pool(name="wstream", bufs=3) as w_p, \
                 tc.tile_pool(name="aux_ps", bufs=1, space="PSUM") as aux_ps, \
                 tc.tile_pool(name="big_ps", bufs=2, space="PSUM") as big_ps, \
                 tc.tile_pool(name="sc_ps", bufs=2, space="PSUM") as sc_ps, \
                 tc.tile_pool(name="l_ps", bufs=1, space="PSUM") as l_ps:
                # rope tables: [128, 4, S]; rows 0:64 x1-tables, 64:128 x2-tables
                ropes = ropes_p.tile([128, 4, S], F32)
                for t in range(4):
                    nc.scalar.dma_start(ropes[:, t, :], ropet[t])
                qT = qt_p.tile([128, NH_L, S], BF16)
                kT = kt_p.tile([128, NKV_L, S], BF16)
                v_sb = v_p.tile([128, 8, VL], BF16)   # [sk-part, s-tile, 2*HD]

                def rms_rope_transpose(ps_src, col, gst, dst, dslot):
                    """ps_src[:, col:col+128] (tokens x head_dim, fp32 psum) ->
                    RMS-normalized, transposed into dst[:, dslot, gst-slice]."""
                    scratch = norm_p.tile([128, 128], F32, tag="scratch")
                    ssum = rstd_p.tile([128, 1], F32, tag="ssum")
                    nc.scalar.activation(scratch, ps_src[:, col:col + 128],
                                         mybir.ActivationFunctionType.Square,
                                         accum_out=ssum)
                    rstd = rstd_p.tile([128, 1], F32, tag="rstd")
                    nc.scalar.activation(rstd, ssum,
                                         mybir.ActivationFunctionType.Sqrt,
                                         scale=1.0 / HD, bias=eps_t)
                    nc.vector.reciprocal(rstd, rstd)
                    qn = norm_p.tile([128, 128], BF16, tag="qn")
                    nc.scalar.activation(qn, ps_src[:, col:col + 128],
                                         mybir.ActivationFunctionType.Copy,
                                         scale=rstd)
                    tps = aux_ps.tile([128, 128], BF16, tag="aux")
                    nc.tensor.transpose(tps, qn, ident)
                    nc.any.tensor_copy(dst[:, dslot, gst * 128:(gst + 1) * 128], tps)

                def rope_apply(dst_slice, base, s0):
                    # dst_slice: [128, 512] bf16 (d on partitions); base 0=q, 2=k
                    # slots: [base]: rows 0:64 = cos*w1, 64:128 = sin*w2
                    #        [base+1]: rows 0:64 = sin*w1, 64:128 = cos*w2
                    x1 = dst_slice[0:64, :]
                    x2 = dst_slice[64:128, :]
                    t0 = ropes[0:64, base, s0:s0 + 512]
                    t1 = ropes[64:128, base, s0:s0 + 512]
                    t3 = ropes[0:64, base + 1, s0:s0 + 512]
                    t2 = ropes[64:128, base + 1, s0:s0 + 512]
                    a = ropetmp_p.tile([64, 512], F32, tag="a")
                    b = ropetmp_p.tile([64, 512], F32, tag="b")
                    c = ropetmp_p.tile([64, 512], F32, tag="c")
                    d = ropetmp_p.tile([64, 512], F32, tag="d")
                    nc.vector.tensor_mul(a, x1, t0)
                    nc.vector.tensor_mul(b, x2, t1)
                    nc.vector.tensor_mul(c, x2, t2)
                    nc.vector.tensor_mul(d, x1, t3)
                    nc.vector.tensor_tensor(x1, a, b, mybir.AluOpType.subtract)
                    nc.vector.tensor_tensor(x2, c, d, mybir.AluOpType.add)

                for sb in range(2):
                    s0 = sb * 512
                    hT = ht_p.tile([128, 32, 512], BF16, tag="ht")
                    # A. DMA-transpose hidden[s0:s0+512, :] into hT (bf16 XBAR)
                    for st in range(4):
                        nc.scalar.dma_start_transpose(
                            hT[:, :, st * 128:(st + 1) * 128],
                            hidden[s0 + st * 128:s0 + (st + 1) * 128, :])
                    # B/C. QKV matmuls + per-head postprocess
                    for fb in range(3):
                        pss2 = [big_ps.tile([128, 2, 512], F32, tag="big",
                                            name=f"qkv_ps{i}") for i in range(2)]
                        pss = [pss2[i // 2][:, i % 2, :] for i in range(4)]
                        for h in range(32):
                            wt = w_p.tile([128, 512], BF16, tag="w")
                            nc.sync.dma_start(
                                wt, w_qkv[h * 128:(h + 1) * 128,
                                          fb * 512:(fb + 1) * 512])
                            for st in range(4):
                                nc.tensor.matmul(
                                    pss[st], hT[:, h, st * 128:(st + 1) * 128], wt,
                                    start=(h == 0), stop=(h == 31))
                        for st in range(4):
                            gst = sb * 4 + st
                            if fb < 2:
                                for hq in range(4):
                                    rms_rope_transpose(pss[st], hq * 128, gst,
                                                       qT, fb * 4 + hq)
                            else:
                                for kh in range(NKV_L):
                                    rms_rope_transpose(pss[st], kh * 128, gst,
                                                       kT, kh)
                                nc.any.tensor_copy(v_sb[:, gst, :],
                                                   pss[st][:, KL:KL + VL])
                    # D. rope on this s-half
                    for head in range(NH_L):
                        rope_apply(qT[:, head, s0:s0 + 512], 0, s0)
                    for kh in range(NKV_L):
                        rope_apply(kT[:, kh, s0:s0 + 512], 2, s0)
                    # E. attention for sq block sb, all heads
                    n_t = 4 * (sb + 1)
                    for head in range(NH_L):
                        kvh = head // 4
                        pv_t = big_ps.tile([128, 2, 512], F32, tag="big", name="pv")
                        pv = pv_t[:, 0, :]
                        acc = acc_p.tile([128, 512], BF16, tag="acc")
                        for t in range(n_t):
                            k0 = t * 128
                            w0 = max(0, k0 - s0)
                            sc = sc_ps.tile([128, 512], F32, tag="sc")
                            nc.tensor.matmul(
                                sc[:, w0:512],
                                kT[:, kvh, k0:k0 + 128],
                                qT[:, head, s0 + w0:s0 + 512],
                                start=True, stop=True)
                            if k0 >= s0:
                                nc.vector.tensor_add(
                                    sc[:, w0:w0 + 128], sc[:, w0:w0 + 128], maskt)
                            pr = probs_p.tile([128, 512], BF16, tag="pr")
                            nc.scalar.activation(
                                pr[:, w0:512], sc[:, w0:512],
                                mybir.ActivationFunctionType.Exp, scale=SCALE)
                            if t == 0:
                                nc.vector.tensor_copy(acc, pr)
                            else:
                                nc.vector.tensor_add(acc[:, w0:512],
                                                     acc[:, w0:512], pr[:, w0:512])
                            nc.tensor.matmul(
                                pv[:, w0:512],
                                v_sb[:, t, kvh * 128:(kvh + 1) * 128],
                                pr[:, w0:512],
                                start=(t == 0), stop=(t == n_t - 1))
                        lps = l_ps.tile([128, 512], F32, tag="l")
                        nc.tensor.matmul(lps, ones_b, acc, start=True, stop=True)
                        rl = ao_p.tile([128, 512], F32, tag="rl")
                        nc.vector.reciprocal(rl, lps)
                        ao = ao_p.tile([128, 512], BF16, tag="ao")
                        nc.vector.tensor_mul(ao, pv, rl)
                        nc.gpsimd.dma_start(
                            attn_loc[sb][head * 128:(head + 1) * 128, :], ao)
                    # F. AllGather for this s-half
                    nc.gpsimd.collective_compute(
                        "AllGather", mybir.AluOpType.bypass,
                        ins=[attn_loc[sb][:, :]], outs=[attn_gat[sb][:, :]],
                        replica_groups=groups)

            # G. out-projection: out[s, o] = sum_f attn_T[f, s] * w_out[f, o]
            with tc.tile_pool(name="ag", bufs=3) as ag_p, \
                 tc.tile_pool(name="osb", bufs=2) as osb_p, \
                 tc.tile_pool(name="op_ps", bufs=2, space="PSUM") as op_ps:
                for gst in range(8):
                    chunk, stc = gst // 4, gst % 4
                    pso = [op_ps.tile([128, 512], F32, tag="op", name=f"op_ps{i}")
                           for i in range(2)]
                    for ft in range(32):
                        agt = ag_p.tile([128, 128], BF16, tag="agt")
                        nc.sync.dma_start(
                            agt, attn_gat[chunk][ft * 128:(ft + 1) * 128,
                                                 stc * 128:(stc + 1) * 128])
                        for ob in range(2):
                            nc.tensor.matmul(
                                pso[ob], agt, wout_sb[:, ft, ob * 512:(ob + 1) * 512],
                                start=(ft == 0), stop=(ft == 31))
                    for ob in range(2):
                        osb = osb_p.tile([128, 512], F32, tag="osb")
                        nc.any.tensor_copy(osb, pso[ob])
                        nc.gpsimd.dma_start(
                            out[gst * 128:(gst + 1) * 128, ob * 512:(ob + 1) * 512],
                            osb)

    _split_excess_waits(nc)
    return nc


def _split_excess_waits(nc):
    # this walrus build allows only one semaphore wait per instruction;
    # hoist extras onto same-engine NOPs placed just before the instruction
    f = nc.m.functions[0]
    for bb in f.blocks:
        new_insts = []
        for inst in bb.instructions:
            si = inst.sync_info
            if si and si.on_wait and len(si.on_wait) > 1:
                extra = si.on_wait[:-1]
                si.on_wait[:] = si.on_wait[-1:]
                for w in extra:
                    nop = mybir.InstNoOp(
                        name=nc.get_next_instruction_name(), ins=[], outs=[],
                        engine=inst.engine,
                        sync_info=mybir.SyncInfo(on_wait=[w], on_update=[]))
                    nc.register_instruction(nop)
                    new_insts.append(nop)
            new_insts.append(inst)
        bb.instructions[:] = new_insts


def _rope_tables(positions_b, norm_w):
    # two slots [2, 128, S]: slot0 = [cos*w1 ; sin*w2], slot1 = [sin*w1 ; cos*w2]
    inv_freq = (1.0 / (THETA ** (np.arange(0, HD, 2, dtype=np.float32) / HD))
                ).astype(np.float32)
    ang = positions_b.astype(np.float32)[:, None] * inv_freq[None, :]
    cos = np.cos(ang).astype(np.float32)      # [S, 64]
    sin = np.sin(ang).astype(np.float32)
    w1 = norm_w[:64].astype(np.float32)
    w2 = norm_w[64:].astype(np.float32)
    slot0 = np.concatenate([(cos * w1[None, :]).T, (sin * w2[None, :]).T], axis=0)
    slot1 = np.concatenate([(sin * w1[None, :]).T, (cos * w2[None, :]).T], axis=0)
    return np.stack([slot0, slot1], axis=0).astype(np.float32)  # [2, 128, S]


def kernel(hidden_states, positions, w_qkv, w_out, q_norm_w, k_norm_w):
    global _CACHED
    if _CACHED is None:
        _CACHED = _build()
    nc = _CACHED

    in_maps = []
    for core in range(8):
        b, g = core // 4, core % 4
        w_local = np.ascontiguousarray(np.concatenate([
            w_qkv[:, g * QL:(g + 1) * QL],
            w_qkv[:, NH * HD + g * KL:NH * HD + (g + 1) * KL],
            w_qkv[:, NH * HD + NKV * HD + g * VL:NH * HD + NKV * HD + (g + 1) * VL],
        ], axis=1)).astype(ml_dtypes.bfloat16)
        wout_local = np.ascontiguousarray(
            w_out[:, g * OL:(g + 1) * OL]).astype(ml_dtypes.bfloat16)
        tq = _rope_tables(np.asarray(positions[b]), np.asarray(q_norm_w))
        tk = _rope_tables(np.asarray(positions[b]), np.asarray(k_norm_w))
        ropet = np.concatenate([tq, tk], axis=0)  # [4, 128, S]
        in_maps.append({
            "hidden": np.ascontiguousarray(hidden_states[b]).astype(ml_dtypes.bfloat16),
            "w_qkv": w_local,
            "w_out": wout_local,
            "ropet": np.ascontiguousarray(ropet),
        })

    res = run_bass_kernel_spmd(nc, in_maps, core_ids=list(range(8)))

    outp = np.empty((B, S, H), dtype=np.float32)
    for core in range(8):
        b, g = core // 4, core % 4
        outp[b, :, g * OL:(g + 1) * OL] = res.results[core]["out"]
    return outp
